# revision 10
# baseline (speedup 1.0000x reference)
"""GPTQ int4 column-parallel linear: y = x @ dequant(qweight) + bias on 8 TRN2 cores.

Sharding: column-parallel over out_features (512 per core). x replicated.
Per-core device kernel:
  - dequantize the int4 weight shard into 32 resident SBUF tiles [128, 512] f32
    (k-order: tile (t, j) holds rows k = 8*kk + j, kk in [128t, 128t+128))
  - stream x in 64 token tiles [128, 4096]; PE-transpose strided k-slices to
    build xT tiles [128k, 128tok]; accumulate 32 f32r matmuls into PSUM
  - add bias, DMA out. Host concatenates the 8 [8192, 512] shards.
"""

import numpy as np

import concourse.bass as bass
import concourse.mybir as mybir
import concourse.tile as tile
from concourse import bacc
from concourse.bass_utils import run_bass_kernel_spmd
from concourse.masks import make_identity

F32 = mybir.dt.float32
F32R = mybir.dt.float32r
I32 = mybir.dt.int32

N_CORES = 8
TOK = 8192
IN_F = 4096
OUT_F = 4096
OUT_SH = OUT_F // N_CORES  # 512
PACKED_K = IN_F // 8  # 512 packed rows
GROUPSIZE = 128
N_GROUPS = IN_F // GROUPSIZE  # 32
P = 128  # partitions

ALU = mybir.AluOpType


def build_nc(tok=TOK):
    n_mtiles = tok // P
    n_ktiles_packed = PACKED_K // P  # 4 tiles of packed rows
    nc = bacc.Bacc(None, target_bir_lowering=False)

    x = nc.dram_tensor("x", [tok, IN_F], F32, kind="ExternalInput")
    qw = nc.dram_tensor("qw", [PACKED_K, OUT_SH], I32, kind="ExternalInput")
    qz = nc.dram_tensor("qz", [N_GROUPS, OUT_SH // 8], I32, kind="ExternalInput")
    sc = nc.dram_tensor("sc", [N_GROUPS, OUT_SH], F32, kind="ExternalInput")
    bi = nc.dram_tensor("bi", [1, OUT_SH], F32, kind="ExternalInput")
    out = nc.dram_tensor("out", [tok, OUT_SH], F32, kind="ExternalOutput")

    with tile.TileContext(nc) as tc:
        with (
            tc.tile_pool(name="singles", bufs=1) as singles,
            tc.tile_pool(name="weights", bufs=1) as wpool,
            tc.tile_pool(name="dq", bufs=2) as dqpool,
            tc.tile_pool(name="xin", bufs=3) as xpool,
            tc.tile_pool(name="xt", bufs=6) as xtpool,
            tc.tile_pool(name="yout", bufs=3) as ypool,
            tc.tile_pool(name="psum_y", bufs=2, space="PSUM") as psum_y,
            tc.tile_pool(name="psum_t", bufs=4, space="PSUM") as psum_t,
            tc.tile_pool(name="dram", bufs=1, space="DRAM") as drampool,
        ):
            # ---- constants ----
            ident_f = singles.tile([P, P], F32)
            make_identity(nc, ident_f)
            ident = singles.tile([P, P], F32R)
            nc.vector.tensor_copy(ident, ident_f)
            bias_sb = singles.tile([P, OUT_SH], F32)
            nc.gpsimd.dma_start(out=bias_sb, in_=bi[:, :].to_broadcast((P, OUT_SH)))

            # ---- zero-point prep: szp[g, n] = scales[g, n] * (zq[g, n] + 1) ----
            qz_sb = singles.tile([N_GROUPS, OUT_SH // 8], I32)
            nc.sync.dma_start(qz_sb, qz[:, :])
            sc_sb = singles.tile([N_GROUPS, OUT_SH], F32)
            nc.sync.dma_start(sc_sb, sc[:, :])
            zq_i = singles.tile([N_GROUPS, OUT_SH], I32)
            zq_i_r = zq_i.rearrange("g (m j) -> g m j", j=8)
            for j in range(8):
                nc.vector.tensor_scalar(
                    out=zq_i_r[:, :, j],
                    in0=qz_sb[:, :],
                    scalar1=4 * j,
                    scalar2=0xF,
                    op0=ALU.logical_shift_right,
                    op1=ALU.bitwise_and,
                )
            zq1 = singles.tile([N_GROUPS, OUT_SH], F32)
            nc.vector.tensor_copy(zq1, zq_i)
            szp = singles.tile([N_GROUPS, OUT_SH], F32)
            nc.vector.scalar_tensor_tensor(
                out=szp,
                in0=zq1,
                scalar=1.0,
                in1=sc_sb,
                op0=ALU.add,
                op1=ALU.mult,
            )
            szp_dram = drampool.tile([N_GROUPS, OUT_SH], F32)
            nc.sync.dma_start(szp_dram[:, :], szp)

            # ---- dequantize weight shard into 32 resident tiles ----
            w_tiles = []
            for t in range(n_ktiles_packed):
                # scale_exp[kk, n] = scales[8t + kk//16, n]; same for szp
                scale_exp = singles.tile([P, OUT_SH], F32R, tag=f"scale_exp{t}")
                nc.sync.dma_start(
                    out=scale_exp,
                    in_=bass.AP(
                        tensor=sc,
                        offset=t * 8 * OUT_SH,
                        ap=[[OUT_SH, 8], [0, 16], [1, OUT_SH]],
                    ).bitcast(F32R),
                )
                szp_exp = singles.tile([P, OUT_SH], F32R, tag=f"szp_exp{t}")
                nc.sync.dma_start(
                    out=szp_exp,
                    in_=bass.AP(
                        tensor=szp_dram.tensor,
                        offset=szp_dram.offset + t * 8 * OUT_SH,
                        ap=[[OUT_SH, 8], [0, 16], [1, OUT_SH]],
                    ).bitcast(F32R),
                )
                qw_t = dqpool.tile([P, OUT_SH], I32, tag="qw")
                nc.sync.dma_start(qw_t, qw[t * P : (t + 1) * P, :])
                for j in range(8):
                    kt = t * 8 + j
                    nib = dqpool.tile([P, OUT_SH], I32, tag="nib")
                    nc.vector.tensor_scalar(
                        out=nib,
                        in0=qw_t,
                        scalar1=4 * j,
                        scalar2=0xF,
                        op0=ALU.logical_shift_right,
                        op1=ALU.bitwise_and,
                    )
                    w = wpool.tile([P, OUT_SH], F32R, tag=f"w{kt}")
                    nc.vector.tensor_copy(w, nib)  # int32 -> f32 cast
                    nc.vector.tensor_mul(w, w, scale_exp)
                    nc.vector.tensor_sub(w, w, szp_exp)
                    w_tiles.append(w)

            # ---- main loop: 64 token tiles ----
            n_kt = n_ktiles_packed * 8
            for m in range(n_mtiles):
                x_sb = xpool.tile([P, IN_F], F32R, tag="x")
                nc.sync.dma_start(x_sb, x[m * P : (m + 1) * P, :].bitcast(F32R))
                # x_r[p, t, i, j] = x[p, t*1024 + i*8 + j]
                x_r = x_sb.rearrange("p (t i j) -> p t i j", t=n_ktiles_packed, j=8)

                ypsum = psum_y.tile([P, OUT_SH], F32, tag="y")

                # software pipeline: transpose one step ahead of matmul
                pts = {}
                xts = {}

                def issue_transpose(kt):
                    t, j = divmod(kt, 8)
                    pt = psum_t.tile([P, P], F32, tag="pt")
                    nc.tensor.transpose(
                        pt.bitcast(F32R), x_r[:, t, :, j], ident
                    )
                    xt = xtpool.tile([P, P], F32R, tag="xt")
                    # alternate PSUM->SBUF copy between scalar and vector engines
                    if kt % 2 == 0:
                        nc.scalar.copy(xt, pt)
                    else:
                        nc.vector.tensor_copy(xt, pt)
                    pts[kt] = pt
                    xts[kt] = xt

                issue_transpose(0)
                for kt in range(n_kt):
                    if kt + 1 < n_kt:
                        issue_transpose(kt + 1)
                    nc.tensor.matmul(
                        ypsum,
                        lhsT=xts[kt],
                        rhs=w_tiles[kt],
                        start=(kt == 0),
                        stop=(kt == n_kt - 1),
                    )

                y_sb = ypool.tile([P, OUT_SH], F32, tag="y_sb")
                nc.vector.tensor_add(y_sb, ypsum, bias_sb)
                nc.sync.dma_start(out[m * P : (m + 1) * P, :], y_sb)

    nc.compile()
    return nc


_NC_CACHE = {}


def _get_nc(tok=TOK):
    if tok not in _NC_CACHE:
        _NC_CACHE[tok] = build_nc(tok)
    return _NC_CACHE[tok]


def _shard_inputs(x, qweight, qzeros, scales, bias):
    in_maps = []
    for c in range(N_CORES):
        sl = slice(c * OUT_SH, (c + 1) * OUT_SH)
        slz = slice(c * (OUT_SH // 8), (c + 1) * (OUT_SH // 8))
        in_maps.append(
            {
                "x": np.ascontiguousarray(x, dtype=np.float32),
                "qw": np.ascontiguousarray(qweight[:, sl], dtype=np.int32),
                "qz": np.ascontiguousarray(qzeros[:, slz], dtype=np.int32),
                "sc": np.ascontiguousarray(scales[:, sl], dtype=np.float32),
                "bi": np.ascontiguousarray(
                    bias[sl].reshape(1, OUT_SH), dtype=np.float32
                ),
            }
        )
    return in_maps


def _kernel_np_fallback(x, qweight, qzeros, scales, g_idx, bias):
    shifts = (np.arange(8, dtype=np.int64) * 4)[None, :, None]
    wq = ((qweight.astype(np.int64)[:, None, :] >> shifts) & 0xF).reshape(
        IN_F, qweight.shape[1]
    )
    zq = (
        (qzeros.astype(np.int64)[:, :, None] >> shifts.reshape(1, 1, 8)) & 0xF
    ).reshape(qzeros.shape[0], -1) + 1
    w = scales[g_idx] * (wq.astype(np.float32) - zq[g_idx].astype(np.float32))
    return (x.astype(np.float32) @ w + bias).astype(np.float32)


class PjrtRunner:
    """Builds the shard_map'd bass executable once; supports timed re-runs."""

    def __init__(self, nc):
        import jax
        from jax.sharding import Mesh, PartitionSpec
        from jax.experimental.shard_map import shard_map
        from concourse import bass2jax, mybir as mb

        self.jax = jax
        bass2jax.install_neuronx_cc_hook()

        partition_name = (
            nc.partition_id_tensor.name if nc.partition_id_tensor else None
        )
        in_names, out_names, out_avals, zero_outs = [], [], [], []
        for alloc in nc.m.functions[0].allocations:
            if not isinstance(alloc, mb.MemoryLocationSet):
                continue
            name = alloc.memorylocations[0].name
            if alloc.kind == "ExternalInput":
                if name != partition_name:
                    in_names.append(name)
            elif alloc.kind == "ExternalOutput":
                shape = tuple(alloc.tensor_shape)
                dtype = mb.dt.np(alloc.dtype)
                out_names.append(name)
                out_avals.append(jax.core.ShapedArray(shape, dtype))
                zero_outs.append(np.zeros(shape, dtype))
        self.in_names = in_names
        self.out_names = out_names
        self.zero_outs = zero_outs
        n_params = len(in_names)
        all_in_names = in_names + out_names
        if partition_name is not None:
            all_in_names.append(partition_name)

        def _body(*args):
            operands = list(args)
            if partition_name is not None:
                operands.append(bass2jax.partition_id_tensor())
            outs = bass2jax._bass_exec_p.bind(
                *operands,
                out_avals=tuple(out_avals),
                in_names=tuple(all_in_names),
                out_names=tuple(out_names),
                lowering_input_output_aliases=(),
                sim_require_finite=True,
                sim_require_nnan=True,
                nc=nc,
            )
            return tuple(outs)

        devices = jax.devices()[:N_CORES]
        self.mesh = Mesh(np.asarray(devices), ("core",))
        in_specs = (PartitionSpec("core"),) * (n_params + len(out_names))
        out_specs = (PartitionSpec("core"),) * len(out_names)
        # no donation: lets us re-run with the same device-resident inputs
        self.fn = jax.jit(
            shard_map(
                _body,
                mesh=self.mesh,
                in_specs=in_specs,
                out_specs=out_specs,
                check_rep=False,
            ),
            keep_unused=True,
        )
        self.out_avals = out_avals

    def stage_inputs(self, in_maps):
        import jax
        from jax.sharding import NamedSharding, PartitionSpec

        sharding = NamedSharding(self.mesh, PartitionSpec("core"))
        args = []
        for i, name in enumerate(self.in_names):
            concat = np.concatenate(
                [np.asarray(m[name]) for m in in_maps], axis=0
            )
            args.append(jax.device_put(concat, sharding))
        for z in self.zero_outs:
            zc = np.zeros((N_CORES * z.shape[0], *z.shape[1:]), z.dtype)
            args.append(jax.device_put(zc, sharding))
        self.args = args

    def run(self):
        outs = self.fn(*self.args)
        self.jax.block_until_ready(outs)
        return outs

    def outputs_to_numpy(self, outs):
        per_core = []
        for c in range(N_CORES):
            per_core.append(
                {
                    name: np.asarray(outs[i]).reshape(
                        N_CORES, *self.out_avals[i].shape
                    )[c]
                    for i, name in enumerate(self.out_names)
                }
            )
        return per_core


_RUNNER_CACHE = {}


def get_runner(tok=TOK):
    if tok not in _RUNNER_CACHE:
        _RUNNER_CACHE[tok] = PjrtRunner(_get_nc(tok))
    return _RUNNER_CACHE[tok]


def kernel(x, qweight, qzeros, scales, g_idx, bias):
    x = np.asarray(x)
    qweight = np.asarray(qweight)
    qzeros = np.asarray(qzeros)
    scales = np.asarray(scales)
    g_idx = np.asarray(g_idx)
    bias = np.asarray(bias)

    if not np.array_equal(
        g_idx, (np.arange(IN_F, dtype=np.int64) // GROUPSIZE).astype(g_idx.dtype)
    ):
        return _kernel_np_fallback(x, qweight, qzeros, scales, g_idx, bias)

    runner = get_runner()
    in_maps = _shard_inputs(x, qweight, qzeros, scales, bias)
    runner.stage_inputs(in_maps)
    outs = runner.run()
    per_core = runner.outputs_to_numpy(outs)
    return np.concatenate([r["out"] for r in per_core], axis=1)


if __name__ == "__main__":
    rng = np.random.default_rng(0)
    x = rng.standard_normal((TOK, IN_F), dtype=np.float32)
    qweight = rng.integers(0, 2**31 - 1, (PACKED_K, OUT_F), dtype=np.int32)
    qzeros = rng.integers(0, 2**31 - 1, (N_GROUPS, OUT_F // 8), dtype=np.int32)
    scales = (rng.random((N_GROUPS, OUT_F), dtype=np.float32) * 0.01).astype(
        np.float32
    )
    g_idx = (np.arange(IN_F) // GROUPSIZE).astype(np.int32)
    bias = (rng.standard_normal(OUT_F).astype(np.float32) * 0.01).astype(np.float32)
    got = kernel(x, qweight, qzeros, scales, g_idx, bias)
    want = _kernel_np_fallback(x, qweight, qzeros, scales, g_idx, bias)
    err = np.abs(got - want).max() / np.abs(want).max()
    print("rel err:", err)


# revision 27
# speedup vs baseline: 88.3216x; 88.3216x over previous
"""GPTQ int4 quant linear: y = x @ dequant(qweight) + bias on 8 TRN2 cores.

Sharding: 2-way over tokens x 4-way over out_features (core c = (ti, oj)).
Each core: x shard [4096, 4096] (67 MB), weight shard [4096k, 1024n].
The 2-way token split halves the PE-transpose count per core, and 1024 local
out_features let each transposed-x tile feed two N=512 matmuls — the
LDWEIGHTS chain (188 ns per f32r 128x128 load) is the PE critical path.

Per-core device kernel:
  - dequantize int4 shard into 32 resident SBUF tiles [128, 1024] f32r
    (tile (t, j) holds W rows k = 8*kk + j, kk in [128t, 128t+128));
    gpsimd unpacks nibbles in place, DVE applies scale/zero-point
  - stream x in 32 token tiles; PE-transpose strided k-slices into
    xT tiles [128k, 128tok]; 2 accumulating f32r matmuls per k-tile
  - add bias, DMA out. Host assembles the 2x4 output grid.
"""

import numpy as np

import concourse.bass as bass
import concourse.mybir as mybir
import concourse.tile as tile
from concourse import bacc

F32 = mybir.dt.float32
F32R = mybir.dt.float32r
I32 = mybir.dt.int32

N_CORES = 8
N_TOK_SHARDS = 2
N_OUT_SHARDS = 4
TOK = 8192
IN_F = 4096
OUT_F = 4096
TOK_SH = TOK // N_TOK_SHARDS  # 4096
OUT_SH = OUT_F // N_OUT_SHARDS  # 1024
PACKED_K = IN_F // 8  # 512 packed rows
GROUPSIZE = 128
N_GROUPS = IN_F // GROUPSIZE  # 32
P = 128

ALU = mybir.AluOpType


def build_nc(tok=TOK_SH):
    n_mtiles = tok // P
    n_t = PACKED_K // P  # 4 packed-row tiles -> 4 chunks of 1024 k
    n_kt = n_t * 8
    nc = bacc.Bacc(None, target_bir_lowering=False)

    x = nc.dram_tensor("x", [tok, IN_F], F32, kind="ExternalInput")
    qw = nc.dram_tensor("qw", [PACKED_K, OUT_SH], I32, kind="ExternalInput")
    qz = nc.dram_tensor("qz", [N_GROUPS, OUT_SH // 8], I32, kind="ExternalInput")
    sc = nc.dram_tensor("sc", [N_GROUPS, OUT_SH], F32, kind="ExternalInput")
    bi = nc.dram_tensor("bi", [1, OUT_SH], F32, kind="ExternalInput")
    out = nc.dram_tensor("out", [tok, OUT_SH], F32, kind="ExternalOutput")

    with tile.TileContext(nc) as tc:
        with (
            tc.tile_pool(name="singles", bufs=1) as singles,
            tc.tile_pool(name="weights", bufs=1) as wpool,
            tc.tile_pool(name="dq", bufs=2) as dqpool,
            tc.tile_pool(name="scexp", bufs=2) as scpool,
            tc.tile_pool(name="xin", bufs=5) as xpool,
            tc.tile_pool(name="xt", bufs=6) as xtpool,
            tc.tile_pool(name="yout", bufs=2) as ypool,
            tc.tile_pool(name="psum_y", bufs=2, space="PSUM") as psum_y,
            tc.tile_pool(name="psum_t", bufs=4, space="PSUM") as psum_t,
            tc.tile_pool(name="dram", bufs=1, space="DRAM") as drampool,
        ):
            # ---- constants ----
            ident_dram = nc.inline_tensor(np.eye(P, dtype=np.float32), name="ident")
            ident = singles.tile([P, P], F32R)
            nc.sync.dma_start(ident, ident_dram[:, :].bitcast(F32R))
            bias_sb = singles.tile([P, OUT_SH], F32)
            nc.gpsimd.dma_start(out=bias_sb, in_=bi[:, :].to_broadcast((P, OUT_SH)))

            # tiny inputs first: the szp chain is on the critical path to W[0]
            qz_sb = singles.tile([N_GROUPS, OUT_SH // 8], I32)
            nc.sync.dma_start(qz_sb, qz[:, :])
            sc_sb = singles.tile([N_GROUPS, OUT_SH], F32)
            nc.sync.dma_start(sc_sb, sc[:, :])

            # x chunk loads (global so the first block's can be hoisted)
            x_r = {}

            def load_chunk(mi, t):
                x_t = xpool.tile([P, 8 * P], F32R, tag="x")
                nc.sync.dma_start(
                    x_t,
                    x[mi * P : (mi + 1) * P, t * 8 * P : (t + 1) * 8 * P].bitcast(
                        F32R
                    ),
                )
                x_r[(mi, t)] = x_t.rearrange("p (i j) -> p i j", j=8)

            for mi in range(min(2, n_mtiles)):
                load_chunk(mi, 0)

            # prefetch weight shard DMAs first so dequant starts ASAP
            qw_tiles = []
            for t in range(n_t):
                qw_t = dqpool.tile([P, OUT_SH], I32, tag="qw")
                nc.sync.dma_start(qw_t, qw[t * P : (t + 1) * P, :])
                qw_tiles.append(qw_t)

            # ---- zero-point prep: szp[g, n] = scales[g, n] * (zq[g, n] + 1) ----
            szp = singles.tile([N_GROUPS, OUT_SH], F32)
            szp_i_r = szp.bitcast(I32).rearrange("g (m j) -> g m j", j=8)
            for j in range(8):
                nc.vector.tensor_scalar(
                    out=szp_i_r[:, :, j],
                    in0=qz_sb[:, :],
                    scalar1=4 * j,
                    scalar2=0xF,
                    op0=ALU.logical_shift_right,
                    op1=ALU.bitwise_and,
                )
            nc.vector.scalar_tensor_tensor(
                out=szp,
                in0=szp.bitcast(I32),
                scalar=1.0,
                in1=sc_sb,
                op0=ALU.add,
                op1=ALU.mult,
            )
            szp_dram = drampool.tile([N_GROUPS, OUT_SH], F32)
            nc.gpsimd.dma_start(szp_dram[:, :], szp)

            # ---- dequantize weight shard into 32 resident tiles ----
            w_tiles = []
            for t in range(n_t):
                # scale_exp[kk, n] = scales[8t + kk//16, n]; same for szp
                scale_exp = scpool.tile([P, OUT_SH], F32R, tag="scale_exp")
                nc.gpsimd.dma_start(
                    out=scale_exp,
                    in_=bass.AP(
                        tensor=sc,
                        offset=t * 8 * OUT_SH,
                        ap=[[OUT_SH, 8], [0, 16], [1, OUT_SH]],
                    ).bitcast(F32R),
                )
                szp_exp = scpool.tile([P, OUT_SH], F32R, tag="szp_exp")
                nc.gpsimd.dma_start(
                    out=szp_exp,
                    in_=bass.AP(
                        tensor=szp_dram.tensor,
                        offset=szp_dram.offset + t * 8 * OUT_SH,
                        ap=[[OUT_SH, 8], [0, 16], [1, OUT_SH]],
                    ).bitcast(F32R),
                )
                qw_t = qw_tiles[t]
                for j in range(8):
                    kt = t * 8 + j
                    nib = dqpool.tile([P, OUT_SH], I32, tag="nib")
                    nc.vector.tensor_scalar(
                        out=nib,
                        in0=qw_t,
                        scalar1=4 * j,
                        scalar2=0xF,
                        op0=ALU.logical_shift_right,
                        op1=ALU.bitwise_and,
                    )
                    w = wpool.tile([P, OUT_SH], F32R, tag=f"w{kt}")
                    nc.vector.tensor_tensor(
                        out=w, in0=nib, in1=scale_exp, op=ALU.mult
                    )
                    nc.vector.tensor_sub(w, w, szp_exp)
                    w_tiles.append(w)

            # ---- main loop: token tiles in pairs, k-major inside a pair ----
            # Interleaving two token tiles keeps the PE fed at 2x rate while
            # the dequant pipeline is still producing W tiles (the first
            # block chases dequant), and gives each PSUM->SBUF xT copy a
            # two-matmul window to hide in.
            # first block is 3 wide so PE consumption (3 x 1.2us per k-tile)
            # keeps up with dequant production (~3us per W tile); pairs after
            blocks = [tuple(range(min(2, n_mtiles)))]
            mnext = blocks[0][-1] + 1
            while mnext < n_mtiles:
                blocks.append(tuple(range(mnext, min(mnext + 2, n_mtiles))))
                mnext += 2
            for ms in blocks:
                mb = ms[0]
                for mi in ms:
                    if (mi, 0) not in x_r:
                        load_chunk(mi, 0)

                ypsums = {}
                for mi in ms:
                    yp = psum_y.tile([P, OUT_SH], F32, tag="y")
                    ypsums[mi] = yp
                xts = {}

                def issue_transpose(mi, kt):
                    t, j = divmod(kt, 8)
                    if j == 0 and (mi, t) not in x_r:
                        load_chunk(mi, t)
                    pt = psum_t.tile([P, P], F32, tag="pt")
                    nc.tensor.transpose(
                        pt.bitcast(F32R), x_r[(mi, t)][:, :, j], ident
                    )
                    xt = xtpool.tile([P, P], F32R, tag="xt")
                    # ScalarE-only while DVE still owns the dequant stream
                    # (FIFO order there would stall the PE behind it);
                    # alternate engines afterwards
                    if mb < 8 or (mi + kt) % 2 == 0:
                        nc.scalar.copy(xt, pt)
                    else:
                        nc.vector.tensor_copy(xt, pt)
                    xts[(mi, kt)] = xt

                for mi in ms:
                    issue_transpose(mi, 0)
                for kt in range(n_kt):
                    for mi in ms:
                        if kt + 1 < n_kt:
                            issue_transpose(mi, kt + 1)
                        for h in range(2):
                            nc.tensor.matmul(
                                ypsums[mi][:, h * 512 : (h + 1) * 512],
                                lhsT=xts[(mi, kt)],
                                rhs=w_tiles[kt][:, h * 512 : (h + 1) * 512],
                                start=(kt == 0),
                                stop=(kt == n_kt - 1),
                            )

                for mi in ms:
                    y_sb = ypool.tile([P, OUT_SH], F32, tag="y_sb")
                    nc.vector.tensor_add(y_sb, ypsums[mi], bias_sb)
                    nc.sync.dma_start(out[mi * P : (mi + 1) * P, :], y_sb)
                for key in [k for k in x_r if k[0] in ms]:
                    del x_r[key]

    nc.compile()
    return nc


_NC_CACHE = {}


def _get_nc(tok=TOK_SH):
    if tok not in _NC_CACHE:
        _NC_CACHE[tok] = build_nc(tok)
    return _NC_CACHE[tok]


def _shard_inputs(x, qweight, qzeros, scales, bias, tok_sh=TOK_SH):
    in_maps = []
    for c in range(N_CORES):
        ti, oj = divmod(c, N_OUT_SHARDS)
        sl = slice(oj * OUT_SH, (oj + 1) * OUT_SH)
        slz = slice(oj * (OUT_SH // 8), (oj + 1) * (OUT_SH // 8))
        in_maps.append(
            {
                "x": np.ascontiguousarray(
                    x[ti * tok_sh : (ti + 1) * tok_sh], dtype=np.float32
                ),
                "qw": np.ascontiguousarray(qweight[:, sl], dtype=np.int32),
                "qz": np.ascontiguousarray(qzeros[:, slz], dtype=np.int32),
                "sc": np.ascontiguousarray(scales[:, sl], dtype=np.float32),
                "bi": np.ascontiguousarray(
                    bias[sl].reshape(1, OUT_SH), dtype=np.float32
                ),
            }
        )
    return in_maps


def _assemble(per_core, tok_sh=TOK_SH):
    out = np.empty((N_TOK_SHARDS * tok_sh, OUT_F), dtype=np.float32)
    for c in range(N_CORES):
        ti, oj = divmod(c, N_OUT_SHARDS)
        out[ti * tok_sh : (ti + 1) * tok_sh, oj * OUT_SH : (oj + 1) * OUT_SH] = (
            per_core[c]["out"]
        )
    return out


_LDW_PATCHED = False


def _enable_ldw_opt():
    """Compile this kernel with walrus's redundant-LDWEIGHTS elimination.

    The two matmuls per (token-tile, k-tile) share the same stationary
    operand; with the default --enable-ldw-opt=false the compiler emits a
    duplicate 190 ns weight load per pair.
    """
    global _LDW_PATCHED
    if _LDW_PATCHED:
        return
    import concourse.bass_utils as bu

    orig_run = bu.run_command

    def run_with_ldw_opt(argv, **kw):
        argv = [
            "--enable-ldw-opt=true" if a == "--enable-ldw-opt=false" else a
            for a in argv
        ]
        return orig_run(argv, **kw)

    bu.run_command = run_with_ldw_opt
    _LDW_PATCHED = True


class PjrtRunner:
    """Builds the shard_map'd bass executable once; supports timed re-runs."""

    def __init__(self, nc):
        import jax
        from jax.sharding import Mesh, PartitionSpec
        from jax.experimental.shard_map import shard_map
        from concourse import bass2jax, mybir as mb

        self.jax = jax
        _enable_ldw_opt()
        bass2jax.install_neuronx_cc_hook()

        partition_name = (
            nc.partition_id_tensor.name if nc.partition_id_tensor else None
        )
        in_names, out_names, out_avals, zero_outs = [], [], [], []
        for alloc in nc.m.functions[0].allocations:
            if not isinstance(alloc, mb.MemoryLocationSet):
                continue
            name = alloc.memorylocations[0].name
            if alloc.kind == "ExternalInput":
                if name != partition_name:
                    in_names.append(name)
            elif alloc.kind == "ExternalOutput":
                shape = tuple(alloc.tensor_shape)
                dtype = mb.dt.np(alloc.dtype)
                out_names.append(name)
                out_avals.append(jax.core.ShapedArray(shape, dtype))
                zero_outs.append(np.zeros(shape, dtype))
        self.in_names = in_names
        self.out_names = out_names
        self.zero_outs = zero_outs
        n_params = len(in_names)
        all_in_names = in_names + out_names
        if partition_name is not None:
            all_in_names.append(partition_name)

        def _body(*args):
            operands = list(args)
            if partition_name is not None:
                operands.append(bass2jax.partition_id_tensor())
            outs = bass2jax._bass_exec_p.bind(
                *operands,
                out_avals=tuple(out_avals),
                in_names=tuple(all_in_names),
                out_names=tuple(out_names),
                lowering_input_output_aliases=(),
                sim_require_finite=True,
                sim_require_nnan=True,
                nc=nc,
            )
            return tuple(outs)

        devices = jax.devices()[:N_CORES]
        self.mesh = Mesh(np.asarray(devices), ("core",))
        in_specs = (PartitionSpec("core"),) * (n_params + len(out_names))
        out_specs = (PartitionSpec("core"),) * len(out_names)
        # no donation: lets us re-run with the same device-resident inputs
        self.fn = jax.jit(
            shard_map(
                _body,
                mesh=self.mesh,
                in_specs=in_specs,
                out_specs=out_specs,
                check_rep=False,
            ),
            keep_unused=True,
        )
        self.out_avals = out_avals

    def stage_inputs(self, in_maps):
        import jax
        from jax.sharding import NamedSharding, PartitionSpec

        sharding = NamedSharding(self.mesh, PartitionSpec("core"))
        args = []
        for name in self.in_names:
            concat = np.concatenate([np.asarray(m[name]) for m in in_maps], axis=0)
            args.append(jax.device_put(concat, sharding))
        for z in self.zero_outs:
            zc = np.zeros((N_CORES * z.shape[0], *z.shape[1:]), z.dtype)
            args.append(jax.device_put(zc, sharding))
        self.args = args

    def run(self):
        outs = self.fn(*self.args)
        self.jax.block_until_ready(outs)
        return outs

    def outputs_to_numpy(self, outs):
        per_core = []
        for c in range(N_CORES):
            per_core.append(
                {
                    name: np.asarray(outs[i]).reshape(
                        N_CORES, *self.out_avals[i].shape
                    )[c]
                    for i, name in enumerate(self.out_names)
                }
            )
        return per_core


_RUNNER_CACHE = {}


def get_runner(tok=TOK_SH):
    if tok not in _RUNNER_CACHE:
        _RUNNER_CACHE[tok] = PjrtRunner(_get_nc(tok))
    return _RUNNER_CACHE[tok]


def _kernel_np_fallback(x, qweight, qzeros, scales, g_idx, bias):
    shifts = (np.arange(8, dtype=np.int64) * 4)[None, :, None]
    wq = ((qweight.astype(np.int64)[:, None, :] >> shifts) & 0xF).reshape(
        IN_F, qweight.shape[1]
    )
    zq = (
        (qzeros.astype(np.int64)[:, :, None] >> shifts.reshape(1, 1, 8)) & 0xF
    ).reshape(qzeros.shape[0], -1) + 1
    w = scales[g_idx] * (wq.astype(np.float32) - zq[g_idx].astype(np.float32))
    return (x.astype(np.float32) @ w + bias).astype(np.float32)


def kernel(x, qweight, qzeros, scales, g_idx, bias):
    x = np.asarray(x)
    qweight = np.asarray(qweight)
    qzeros = np.asarray(qzeros)
    scales = np.asarray(scales)
    g_idx = np.asarray(g_idx)
    bias = np.asarray(bias)

    if not np.array_equal(
        g_idx, (np.arange(IN_F, dtype=np.int64) // GROUPSIZE).astype(g_idx.dtype)
    ):
        return _kernel_np_fallback(x, qweight, qzeros, scales, g_idx, bias)

    runner = get_runner()
    runner.stage_inputs(_shard_inputs(x, qweight, qzeros, scales, bias))
    outs = runner.run()
    return _assemble(runner.outputs_to_numpy(outs))


# revision 29
# speedup vs baseline: 97.0934x; 1.0993x over previous
"""GPTQ int4 quant linear: y = x @ dequant(qweight) + bias on 8 TRN2 cores.

Sharding: 2-way over tokens x 4-way over out_features (core c = (ti, oj)).
Each core: x shard [4096, 4096] (67 MB), weight shard [4096k, 1024n].
The 2-way token split halves the PE-transpose count per core, and 1024 local
out_features let each transposed-x tile feed two N=512 matmuls. Weights and
the transposed activations are bf16 (cast for free inside the dequant ops
and the PSUM->SBUF copies) so the matmul weight loads get fast-weight-load;
the PE transposes read x as float32r. Measured ~655 us/core on silicon,
rel err ~3.5e-3.

Per-core device kernel:
  - dequantize int4 shard into 32 resident SBUF tiles [128, 1024] bf16
    (tile (t, j) holds W rows k = 8*kk + j, kk in [128t, 128t+128))
  - stream x in 32 token tiles; PE-transpose strided k-slices into
    xT tiles [128k, 128tok] (bf16); 2 accumulating matmuls per k-tile
  - add bias, DMA out. Host assembles the 2x4 output grid.
"""

import numpy as np

import concourse.bass as bass
import concourse.mybir as mybir
import concourse.tile as tile
from concourse import bacc

F32 = mybir.dt.float32
F32R = mybir.dt.float32r
I32 = mybir.dt.int32
BF16 = mybir.dt.bfloat16

N_CORES = 8
N_TOK_SHARDS = 2
N_OUT_SHARDS = 4
TOK = 8192
IN_F = 4096
OUT_F = 4096
TOK_SH = TOK // N_TOK_SHARDS  # 4096
OUT_SH = OUT_F // N_OUT_SHARDS  # 1024
PACKED_K = IN_F // 8  # 512 packed rows
GROUPSIZE = 128
N_GROUPS = IN_F // GROUPSIZE  # 32
P = 128

ALU = mybir.AluOpType


def build_nc(tok=TOK_SH):
    n_mtiles = tok // P
    n_t = PACKED_K // P  # 4 packed-row tiles -> 4 chunks of 1024 k
    n_kt = n_t * 8
    nc = bacc.Bacc(None, target_bir_lowering=False)

    x = nc.dram_tensor("x", [tok, IN_F], F32, kind="ExternalInput")
    qw = nc.dram_tensor("qw", [PACKED_K, OUT_SH], I32, kind="ExternalInput")
    qz = nc.dram_tensor("qz", [N_GROUPS, OUT_SH // 8], I32, kind="ExternalInput")
    sc = nc.dram_tensor("sc", [N_GROUPS, OUT_SH], F32, kind="ExternalInput")
    bi = nc.dram_tensor("bi", [1, OUT_SH], F32, kind="ExternalInput")
    out = nc.dram_tensor("out", [tok, OUT_SH], F32, kind="ExternalOutput")

    with tile.TileContext(nc) as tc:
        with (
            tc.tile_pool(name="singles", bufs=1) as singles,
            tc.tile_pool(name="weights", bufs=1) as wpool,
            tc.tile_pool(name="dq", bufs=2) as dqpool,
            tc.tile_pool(name="scexp", bufs=2) as scpool,
            tc.tile_pool(name="xin", bufs=5) as xpool,
            tc.tile_pool(name="xt", bufs=6) as xtpool,
            tc.tile_pool(name="yout", bufs=2) as ypool,
            tc.tile_pool(name="psum_y", bufs=2, space="PSUM") as psum_y,
            tc.tile_pool(name="psum_t", bufs=4, space="PSUM") as psum_t,
            tc.tile_pool(name="dram", bufs=1, space="DRAM") as drampool,
        ):
            # ---- constants ----
            ident_dram = nc.inline_tensor(np.eye(P, dtype=np.float32), name="ident")
            ident = singles.tile([P, P], F32R)
            nc.sync.dma_start(ident, ident_dram[:, :].bitcast(F32R))
            bias_sb = singles.tile([P, OUT_SH], F32)
            nc.gpsimd.dma_start(out=bias_sb, in_=bi[:, :].to_broadcast((P, OUT_SH)))

            # tiny inputs first: the szp chain is on the critical path to W[0]
            qz_sb = singles.tile([N_GROUPS, OUT_SH // 8], I32)
            nc.sync.dma_start(qz_sb, qz[:, :])
            sc_sb = singles.tile([N_GROUPS, OUT_SH], F32)
            nc.sync.dma_start(sc_sb, sc[:, :])

            # x chunk loads (global so the first block's can be hoisted)
            x_r = {}

            def load_chunk(mi, t):
                x_t = xpool.tile([P, 8 * P], F32R, tag="x")
                nc.sync.dma_start(
                    x_t,
                    x[mi * P : (mi + 1) * P, t * 8 * P : (t + 1) * 8 * P].bitcast(
                        F32R
                    ),
                )
                x_r[(mi, t)] = x_t.rearrange("p (i j) -> p i j", j=8)

            for mi in range(min(2, n_mtiles)):
                load_chunk(mi, 0)

            # prefetch weight shard DMAs first so dequant starts ASAP
            qw_tiles = []
            for t in range(n_t):
                qw_t = dqpool.tile([P, OUT_SH], I32, tag="qw")
                nc.sync.dma_start(qw_t, qw[t * P : (t + 1) * P, :])
                qw_tiles.append(qw_t)

            # ---- zero-point prep: szp[g, n] = scales[g, n] * (zq[g, n] + 1) ----
            szp = singles.tile([N_GROUPS, OUT_SH], F32)
            szp_i_r = szp.bitcast(I32).rearrange("g (m j) -> g m j", j=8)
            for j in range(8):
                nc.vector.tensor_scalar(
                    out=szp_i_r[:, :, j],
                    in0=qz_sb[:, :],
                    scalar1=4 * j,
                    scalar2=0xF,
                    op0=ALU.logical_shift_right,
                    op1=ALU.bitwise_and,
                )
            nc.vector.scalar_tensor_tensor(
                out=szp,
                in0=szp.bitcast(I32),
                scalar=1.0,
                in1=sc_sb,
                op0=ALU.add,
                op1=ALU.mult,
            )
            szp_dram = drampool.tile([N_GROUPS, OUT_SH], F32)
            nc.gpsimd.dma_start(szp_dram[:, :], szp)

            # ---- dequantize weight shard into 32 resident tiles ----
            w_tiles = []
            for t in range(n_t):
                # scale_exp[kk, n] = scales[8t + kk//16, n]; same for szp
                scale_exp = scpool.tile([P, OUT_SH], F32, tag="scale_exp")
                nc.gpsimd.dma_start(
                    out=scale_exp,
                    in_=bass.AP(
                        tensor=sc,
                        offset=t * 8 * OUT_SH,
                        ap=[[OUT_SH, 8], [0, 16], [1, OUT_SH]],
                    ),
                )
                szp_exp = scpool.tile([P, OUT_SH], F32, tag="szp_exp")
                nc.gpsimd.dma_start(
                    out=szp_exp,
                    in_=bass.AP(
                        tensor=szp_dram.tensor,
                        offset=szp_dram.offset + t * 8 * OUT_SH,
                        ap=[[OUT_SH, 8], [0, 16], [1, OUT_SH]],
                    ),
                )
                qw_t = qw_tiles[t]
                for j in range(8):
                    kt = t * 8 + j
                    nib = dqpool.tile([P, OUT_SH], I32, tag="nib")
                    nc.vector.tensor_scalar(
                        out=nib,
                        in0=qw_t,
                        scalar1=4 * j,
                        scalar2=0xF,
                        op0=ALU.logical_shift_right,
                        op1=ALU.bitwise_and,
                    )
                    w = wpool.tile([P, OUT_SH], BF16, tag=f"w{kt}")
                    nc.vector.tensor_tensor(
                        out=w, in0=nib, in1=scale_exp, op=ALU.mult
                    )
                    nc.vector.tensor_sub(w, w, szp_exp)
                    w_tiles.append(w)

            # ---- main loop: token tiles in pairs, k-major inside a pair ----
            # Interleaving two token tiles keeps the PE fed at 2x rate while
            # the dequant pipeline is still producing W tiles (the first
            # block chases dequant), and gives each PSUM->SBUF xT copy a
            # two-matmul window to hide in.
            blocks = [tuple(range(min(2, n_mtiles)))]
            mnext = blocks[0][-1] + 1
            while mnext < n_mtiles:
                blocks.append(tuple(range(mnext, min(mnext + 2, n_mtiles))))
                mnext += 2
            for ms in blocks:
                mb = ms[0]
                for mi in ms:
                    if (mi, 0) not in x_r:
                        load_chunk(mi, 0)

                ypsums = {}
                for mi in ms:
                    yp = psum_y.tile([P, OUT_SH], F32, tag="y")
                    ypsums[mi] = yp
                xts = {}

                def issue_transpose(mi, kt):
                    t, j = divmod(kt, 8)
                    if j == 0 and (mi, t) not in x_r:
                        load_chunk(mi, t)
                    pt = psum_t.tile([P, P], F32, tag="pt")
                    nc.tensor.transpose(
                        pt.bitcast(F32R), x_r[(mi, t)][:, :, j], ident
                    )
                    xt = xtpool.tile([P, P], BF16, tag="xt")
                    # ScalarE-only while DVE still owns the dequant stream
                    # (FIFO order there would stall the PE behind it);
                    # alternate engines afterwards
                    if mb < 8 or (mi + kt) % 2 == 0:
                        nc.scalar.copy(xt, pt)
                    else:
                        nc.vector.tensor_copy(xt, pt)
                    xts[(mi, kt)] = xt

                for mi in ms:
                    issue_transpose(mi, 0)
                for kt in range(n_kt):
                    for mi in ms:
                        if kt + 1 < n_kt:
                            issue_transpose(mi, kt + 1)
                        for h in range(2):
                            nc.tensor.matmul(
                                ypsums[mi][:, h * 512 : (h + 1) * 512],
                                lhsT=xts[(mi, kt)],
                                rhs=w_tiles[kt][:, h * 512 : (h + 1) * 512],
                                start=(kt == 0),
                                stop=(kt == n_kt - 1),
                            )

                for mi in ms:
                    y_sb = ypool.tile([P, OUT_SH], F32, tag="y_sb")
                    nc.vector.tensor_add(y_sb, ypsums[mi], bias_sb)
                    nc.sync.dma_start(out[mi * P : (mi + 1) * P, :], y_sb)
                for key in [k for k in x_r if k[0] in ms]:
                    del x_r[key]

    nc.compile()
    return nc


_NC_CACHE = {}


def _get_nc(tok=TOK_SH):
    if tok not in _NC_CACHE:
        _NC_CACHE[tok] = build_nc(tok)
    return _NC_CACHE[tok]


def _shard_inputs(x, qweight, qzeros, scales, bias, tok_sh=TOK_SH):
    in_maps = []
    for c in range(N_CORES):
        ti, oj = divmod(c, N_OUT_SHARDS)
        sl = slice(oj * OUT_SH, (oj + 1) * OUT_SH)
        slz = slice(oj * (OUT_SH // 8), (oj + 1) * (OUT_SH // 8))
        in_maps.append(
            {
                "x": np.ascontiguousarray(
                    x[ti * tok_sh : (ti + 1) * tok_sh], dtype=np.float32
                ),
                "qw": np.ascontiguousarray(qweight[:, sl], dtype=np.int32),
                "qz": np.ascontiguousarray(qzeros[:, slz], dtype=np.int32),
                "sc": np.ascontiguousarray(scales[:, sl], dtype=np.float32),
                "bi": np.ascontiguousarray(
                    bias[sl].reshape(1, OUT_SH), dtype=np.float32
                ),
            }
        )
    return in_maps


def _assemble(per_core, tok_sh=TOK_SH):
    out = np.empty((N_TOK_SHARDS * tok_sh, OUT_F), dtype=np.float32)
    for c in range(N_CORES):
        ti, oj = divmod(c, N_OUT_SHARDS)
        out[ti * tok_sh : (ti + 1) * tok_sh, oj * OUT_SH : (oj + 1) * OUT_SH] = (
            per_core[c]["out"]
        )
    return out


class PjrtRunner:
    """Builds the shard_map'd bass executable once; supports timed re-runs."""

    def __init__(self, nc):
        import jax
        from jax.sharding import Mesh, PartitionSpec
        from jax.experimental.shard_map import shard_map
        from concourse import bass2jax, mybir as mb

        self.jax = jax
        bass2jax.install_neuronx_cc_hook()

        partition_name = (
            nc.partition_id_tensor.name if nc.partition_id_tensor else None
        )
        in_names, out_names, out_avals, zero_outs = [], [], [], []
        for alloc in nc.m.functions[0].allocations:
            if not isinstance(alloc, mb.MemoryLocationSet):
                continue
            name = alloc.memorylocations[0].name
            if alloc.kind == "ExternalInput":
                if name != partition_name:
                    in_names.append(name)
            elif alloc.kind == "ExternalOutput":
                shape = tuple(alloc.tensor_shape)
                dtype = mb.dt.np(alloc.dtype)
                out_names.append(name)
                out_avals.append(jax.core.ShapedArray(shape, dtype))
                zero_outs.append(np.zeros(shape, dtype))
        self.in_names = in_names
        self.out_names = out_names
        self.zero_outs = zero_outs
        n_params = len(in_names)
        all_in_names = in_names + out_names
        if partition_name is not None:
            all_in_names.append(partition_name)

        def _body(*args):
            operands = list(args)
            if partition_name is not None:
                operands.append(bass2jax.partition_id_tensor())
            outs = bass2jax._bass_exec_p.bind(
                *operands,
                out_avals=tuple(out_avals),
                in_names=tuple(all_in_names),
                out_names=tuple(out_names),
                lowering_input_output_aliases=(),
                sim_require_finite=True,
                sim_require_nnan=True,
                nc=nc,
            )
            return tuple(outs)

        devices = jax.devices()[:N_CORES]
        self.mesh = Mesh(np.asarray(devices), ("core",))
        in_specs = (PartitionSpec("core"),) * (n_params + len(out_names))
        out_specs = (PartitionSpec("core"),) * len(out_names)
        # no donation: lets us re-run with the same device-resident inputs
        self.fn = jax.jit(
            shard_map(
                _body,
                mesh=self.mesh,
                in_specs=in_specs,
                out_specs=out_specs,
                check_rep=False,
            ),
            keep_unused=True,
        )
        self.out_avals = out_avals

    def stage_inputs(self, in_maps):
        import jax
        from jax.sharding import NamedSharding, PartitionSpec

        sharding = NamedSharding(self.mesh, PartitionSpec("core"))
        args = []
        for name in self.in_names:
            concat = np.concatenate([np.asarray(m[name]) for m in in_maps], axis=0)
            args.append(jax.device_put(concat, sharding))
        for z in self.zero_outs:
            zc = np.zeros((N_CORES * z.shape[0], *z.shape[1:]), z.dtype)
            args.append(jax.device_put(zc, sharding))
        self.args = args

    def run(self):
        outs = self.fn(*self.args)
        self.jax.block_until_ready(outs)
        return outs

    def outputs_to_numpy(self, outs):
        per_core = []
        for c in range(N_CORES):
            per_core.append(
                {
                    name: np.asarray(outs[i]).reshape(
                        N_CORES, *self.out_avals[i].shape
                    )[c]
                    for i, name in enumerate(self.out_names)
                }
            )
        return per_core


_RUNNER_CACHE = {}


def get_runner(tok=TOK_SH):
    if tok not in _RUNNER_CACHE:
        _RUNNER_CACHE[tok] = PjrtRunner(_get_nc(tok))
    return _RUNNER_CACHE[tok]


def _kernel_np_fallback(x, qweight, qzeros, scales, g_idx, bias):
    shifts = (np.arange(8, dtype=np.int64) * 4)[None, :, None]
    wq = ((qweight.astype(np.int64)[:, None, :] >> shifts) & 0xF).reshape(
        IN_F, qweight.shape[1]
    )
    zq = (
        (qzeros.astype(np.int64)[:, :, None] >> shifts.reshape(1, 1, 8)) & 0xF
    ).reshape(qzeros.shape[0], -1) + 1
    w = scales[g_idx] * (wq.astype(np.float32) - zq[g_idx].astype(np.float32))
    return (x.astype(np.float32) @ w + bias).astype(np.float32)


def kernel(x, qweight, qzeros, scales, g_idx, bias):
    x = np.asarray(x)
    qweight = np.asarray(qweight)
    qzeros = np.asarray(qzeros)
    scales = np.asarray(scales)
    g_idx = np.asarray(g_idx)
    bias = np.asarray(bias)

    if not np.array_equal(
        g_idx, (np.arange(IN_F, dtype=np.int64) // GROUPSIZE).astype(g_idx.dtype)
    ):
        return _kernel_np_fallback(x, qweight, qzeros, scales, g_idx, bias)

    runner = get_runner()
    runner.stage_inputs(_shard_inputs(x, qweight, qzeros, scales, bias))
    outs = runner.run()
    return _assemble(runner.outputs_to_numpy(outs))
